# revision 2
# baseline (speedup 1.0000x reference)
"""Trainium2 Bass kernel for nn_CrossAttention (B=8, L=1024, QD=1024, KVD=768, H=16).

Sharding: data-parallel over batch across the 8 NeuronCores (1 batch row each).
Per-core pipeline (all bf16 matmuls, fp32 accumulation / residual / layernorm):
  A) prologue with NO DRAM bounce: fp32->bf16 DMA-cast HBM->SBUF on SWDGE into
     natural row-tile chunks, then SBUF->SBUF xbar transposes (sync HWDGE ring
     ONLY -- concurrent xbar use from both HWDGE rings corrupts data) into
     lt-major transposed layouts [P, rowtile, ct, 128]. Cuts HBM traffic from
     ~46MB to ~32MB/core and lets B1 start ~20us in instead of ~85us.
  B) projections: qhT/khT (transposed world, per-partition bias via
     tensor_scalar), vh natural with bias added on DVE during psum eviction
     (bv broadcast tile). B1 is lh-outer so it can start on half of qT.
  C) attention per head pair: scoresT = khT.T @ qhT, exp with mask+scale folded
     into the ACT pass, attnV with [ones|vh] stationary giving psum rows 0:64 =
     replicated denominator and rows 64:128 = o; fast approx reciprocal +
     multiply on DVE. attnV lags one pair behind scores+exp.
  D) out-projection from oT stationary + rank-1 bias, fp32 residual + layernorm
     with per-512-half eviction/bn_stats to shorten the tail.
"""

import numpy as np

import concourse.bass as bass
import concourse.mybir as mybir
import concourse.tile as tile
from concourse import bacc
from concourse.bass_utils import run_bass_kernel_spmd

F32 = mybir.dt.float32
BF16 = mybir.dt.bfloat16
U8 = mybir.dt.uint8

B = 8
L = 1024
C = 1024      # QD
KV = 768      # KVD
H = 16
DH = 64
P = 128
LT = L // P          # 8 l-tiles
CT = C // P          # 8 contraction tiles (model dim)
KT = KV // P         # 6 contraction tiles (kv dim)
DT = C // P          # 8 d-tiles
NH = C // 512        # 2 free-dim halves (N=512 per PSUM bank)
SCALE = DH ** -0.5
EPS = 1e-5
MASK_NEG = -30000.0

Exp = mybir.ActivationFunctionType.Exp
Sqrt = mybir.ActivationFunctionType.Sqrt
Identity = mybir.ActivationFunctionType.Identity
MULT = mybir.AluOpType.mult
ADD = mybir.AluOpType.add

TRACE = False
LAST_RESULT = None
_CACHE = {}


def _bcast_ap(handle, parts):
    apx = handle[:]
    return bass.AP(tensor=apx.tensor, offset=apx.offset,
                   ap=[[0, parts]] + [list(x) for x in apx.ap])


def _rowtiles(hnd, r0, nt, cols):
    # DRAM AP [p, j, c] = hnd[(j + r0/P)*P + p, c] for j in [0, nt)
    return hnd[r0:r0 + nt * P, :].rearrange("(j p) c -> p j c", p=P)


def build(apply_gb=False):
    nc = bacc.Bacc("TRN2", target_bir_lowering=False)

    q_in = nc.dram_tensor("q", [L, C], F32, kind="ExternalInput")
    k_in = nc.dram_tensor("k", [L, KV], F32, kind="ExternalInput")
    v_in = nc.dram_tensor("v", [L, KV], F32, kind="ExternalInput")
    m_in = nc.dram_tensor("key_padding_mask", [L], U8, kind="ExternalInput")
    wq_in = nc.dram_tensor("Wq", [C, C], F32, kind="ExternalInput")
    bq_in = nc.dram_tensor("bq", [C], F32, kind="ExternalInput")
    wk_in = nc.dram_tensor("Wk", [C, KV], F32, kind="ExternalInput")
    bk_in = nc.dram_tensor("bk", [C], F32, kind="ExternalInput")
    wv_in = nc.dram_tensor("Wv", [C, KV], F32, kind="ExternalInput")
    bv_in = nc.dram_tensor("bv", [C], F32, kind="ExternalInput")
    wo_in = nc.dram_tensor("Wo", [C, C], F32, kind="ExternalInput")
    bo_in = nc.dram_tensor("bo", [C], F32, kind="ExternalInput")
    gamma_in = nc.dram_tensor("gamma", [C], F32, kind="ExternalInput")
    beta_in = nc.dram_tensor("beta", [C], F32, kind="ExternalInput")
    y_out = nc.dram_tensor("y", [L, C], F32, kind="ExternalOutput")

    with tile.TileContext(nc) as tc:
        with (
            tc.tile_pool(name="persist", bufs=1) as persist,
            tc.tile_pool(name="cst", bufs=1) as cst,
            tc.tile_pool(name="wostg", bufs=1) as wostg,
            tc.tile_pool(name="poolV", bufs=1) as poolV,
        ):
            # persistent projection outputs
            qhT = persist.tile([P, DT, L], BF16)          # d on partitions
            khT = persist.tile([P, DT, L], BF16)
            vh_aug = persist.tile([P, LT, H * P], BF16)   # per m-tile: 16x[64 ones | 64 vh]
            WoT = persist.tile([P, DT, CT, P], BF16)      # lt-major transposed Wo

            vT = poolV.tile([P, LT, KT, P], BF16)
            WvT = poolV.tile([P, DT, KT, P], BF16)

            with tc.tile_pool(name="poolK", bufs=1) as poolK:
                kT = poolK.tile([P, LT, KT, P], BF16)
                WkT = poolK.tile([P, DT, KT, P], BF16)

                with (
                    tc.tile_pool(name="stg", bufs=3) as stg,
                    tc.tile_pool(name="poolQ", bufs=1) as poolQ,
                    tc.tile_pool(name="psum_b", bufs=2, space="PSUM") as psum_b,
                ):
                    qT = poolQ.tile([P, LT, CT, P], BF16)
                    WqT = poolQ.tile([P, DT, CT, P], BF16)

                    # tiny consts first (they gate B evictions / first exps)
                    bq_sb = cst.tile([P, DT], F32)
                    nc.gpsimd.dma_start(bq_sb, bq_in[:].rearrange("(t p) -> p t", p=P))
                    bk_sb = cst.tile([P, DT], F32)
                    nc.gpsimd.dma_start(bk_sb, bk_in[:].rearrange("(t p) -> p t", p=P))
                    mask_u8 = cst.tile([P, LT], U8)
                    nc.gpsimd.dma_start(mask_u8, m_in[:].rearrange("(t p) -> p t", p=P))
                    mask_bias = cst.tile([P, LT], F32)
                    nc.vector.tensor_copy(mask_bias, mask_u8)
                    nc.vector.tensor_scalar(mask_bias, mask_bias, -MASK_NEG, MASK_NEG,
                                            MULT, ADD)
                    ones_row = cst.tile([1, P], BF16)
                    nc.vector.memset(ones_row, 1.0)
                    eps_sb = cst.tile([P, 1], F32)
                    nc.vector.memset(eps_sb, EPS)
                    bvb = cst.tile([P, C], F32)
                    nc.gpsimd.dma_start(bvb, _bcast_ap(bv_in, P))
                    bo_bf = cst.tile([1, C], BF16)
                    nc.gpsimd.dma_start(bo_bf, bo_in[:].rearrange("(a c) -> a c", a=1))
                    if apply_gb:
                        gamma_b = cst.tile([P, C], F32)
                        nc.gpsimd.dma_start(gamma_b, _bcast_ap(gamma_in, P))
                        beta_b = cst.tile([P, C], F32)
                        nc.gpsimd.dma_start(beta_b, _bcast_ap(beta_in, P))
                    else:
                        gamma_b = beta_b = None

                    # ---- loads: DMA-cast fp32->bf16 on SWDGE (gpsimd), 4-rowtile
                    # chunks; transposes SBUF->SBUF on the sync HWDGE ring ONLY.
                    def load_mat(nm, hnd, rows, cols, dstT, pool):
                        nt = rows // P
                        for ch in range(nt // 4):
                            st = pool.tile([P, 4, cols], BF16,
                                           name=f"st_{nm}{ch}", tag=f"stg{pool.name}")
                            nc.gpsimd.dma_start(st, _rowtiles(hnd, ch * 4 * P, 4, cols))
                            for j in range(4):
                                jt = ch * 4 + j
                                nc.sync.dma_start(dstT[:, jt, :, :], st[:, j, :],
                                                  transpose=True)

                    load_mat("wq", wq_in, C, C, WqT, stg)
                    load_mat("q", q_in, L, C, qT, stg)
                    load_mat("wk", wk_in, C, KV, WkT, stg)
                    load_mat("k", k_in, L, KV, kT, stg)
                    load_mat("wv", wv_in, C, KV, WvT, stg)
                    load_mat("v", v_in, L, KV, vT, stg)
                    load_mat("wo", wo_in, C, C, WoT, wostg)

                    # ---- B1: qhT[d, l]  (lh-outer: starts on half of qT)
                    for lh in range(NH):
                        for dt in range(DT):
                            ps = psum_b.tile([P, 512], F32, tag="ps")
                            for ct in range(CT):
                                nc.tensor.matmul(ps, WqT[:, dt, ct, :],
                                                 qT[:, lh * 4:(lh + 1) * 4, ct, :],
                                                 start=(ct == 0), stop=(ct == CT - 1))
                            nc.vector.tensor_scalar_add(
                                qhT[:, dt, lh * 512:(lh + 1) * 512], ps,
                                bq_sb[:, dt:dt + 1])

                # ---- B2: khT[d, l]
                with tc.tile_pool(name="psum_b2", bufs=2, space="PSUM") as psum_b2:
                    for lh in range(NH):
                        for dt in range(DT):
                            ps = psum_b2.tile([P, 512], F32, tag="ps")
                            for ct in range(KT):
                                nc.tensor.matmul(ps, WkT[:, dt, ct, :],
                                                 kT[:, lh * 4:(lh + 1) * 4, ct, :],
                                                 start=(ct == 0), stop=(ct == KT - 1))
                            nc.vector.tensor_scalar_add(
                                khT[:, dt, lh * 512:(lh + 1) * 512], ps,
                                bk_sb[:, dt:dt + 1])

            with tc.tile_pool(name="late", bufs=1) as late:
                oT = late.tile([P, DT, L], BF16)

                # ---------------- attention, with B3 (vh projection)
                # interleaved into the first two pair slots
                with (
                    tc.tile_pool(name="ptp", bufs=26) as ptp,
                    tc.tile_pool(name="recp", bufs=4) as recp,
                    tc.tile_pool(name="psum_sc", bufs=2, space="PSUM") as psum_sc,
                    tc.tile_pool(name="psum_av", bufs=3, space="PSUM") as psum_av,
                    tc.tile_pool(name="psum_b3", bufs=1, space="PSUM") as psum_b3,
                ):
                    pts = {}

                    def scores_exp(pair):
                        for mt in range(LT):
                            sc = []
                            for hh in range(2):
                                s = psum_sc.tile([P, L], F32,
                                                 name=f"sc{pair}_{mt}_{hh}", tag="sc")
                                sc.append(s)
                                p0 = hh * DH
                                for lh in range(NH):
                                    nc.tensor.matmul(
                                        s[:, lh * 512:(lh + 1) * 512],
                                        khT[p0:p0 + DH, pair, mt * P:(mt + 1) * P],
                                        qhT[p0:p0 + DH, pair, lh * 512:(lh + 1) * 512],
                                        start=True, stop=True)
                            for hh in range(2):
                                pt = ptp.tile([P, L], BF16,
                                              name=f"pt{pair}_{mt}_{hh}", tag="pt")
                                pts[(pair, mt, hh)] = pt
                                nc.scalar.activation(pt, sc[hh], Exp,
                                                     bias=mask_bias[:, mt:mt + 1],
                                                     scale=SCALE)

                    def b3_chunk(mts):
                        for mt in mts:
                            for dh2 in range(NH):
                                ps = psum_b3.tile([P, 512], F32, tag="ps3")
                                for ct in range(KT):
                                    nc.tensor.matmul(
                                        ps, vT[:, mt, ct, :],
                                        WvT[:, dh2 * 4:(dh2 + 1) * 4, ct, :],
                                        start=(ct == 0), stop=(ct == KT - 1))
                                dst = vh_aug[:, mt, :].rearrange(
                                    "p (h x) -> p h x", x=P)
                                dst = dst[:, dh2 * 8:(dh2 + 1) * 8, DH:P]
                                bvs = bvb[:, dh2 * 512:(dh2 + 1) * 512].rearrange(
                                    "p (h d) -> p h d", d=DH)
                                nc.vector.tensor_add(
                                    dst, ps[:].rearrange("p (h d) -> p h d", d=DH),
                                    bvs)

                    def attnv(pair):
                        for hh in range(2):
                            h = 2 * pair + hh
                            avs = [psum_av.tile([P, 512], F32,
                                                name=f"av{pair}_{hh}_{lh}",
                                                tag="av")
                                   for lh in range(NH)]
                            # mt-outer: both l-halves reuse each vh stationary load
                            for mt in range(LT):
                                for lh in range(NH):
                                    nc.tensor.matmul(
                                        avs[lh],
                                        vh_aug[:, mt, h * P:(h + 1) * P],
                                        pts[(pair, mt, hh)][:, lh * 512:(lh + 1) * 512],
                                        start=(mt == 0), stop=(mt == LT - 1))
                            for lh in range(NH):
                                av = avs[lh]
                                rec = recp.tile([P, 512], F32,
                                                name=f"rec{pair}_{hh}_{lh}",
                                                tag="rec")
                                nc.vector.reciprocal_approx_fast(rec[0:DH, :],
                                                                 av[0:DH, :])
                                nc.vector.tensor_mul(
                                    oT[hh * DH:(hh + 1) * DH, pair,
                                       lh * 512:(lh + 1) * 512],
                                    av[DH:P, :], rec[0:DH, :])
                            for mt in range(LT):
                                del pts[(pair, mt, hh)]

                    nc.vector.memset(vh_aug[:], 1.0)
                    scores_exp(0)
                    b3_chunk(range(0, 4))
                    scores_exp(1)
                    b3_chunk(range(4, 8))
                    attnv(0)
                    for pair in range(2, H // 2):
                        scores_exp(pair)
                        attnv(pair - 1)
                    attnv(H // 2 - 1)

                # ---------------- out-projection + residual + layernorm
                with (
                    tc.tile_pool(name="dwork", bufs=3) as dwork,
                    tc.tile_pool(name="dsmall", bufs=8) as dsmall,
                    tc.tile_pool(name="psum_y", bufs=3, space="PSUM") as psum_y,
                ):
                    qrs = []
                    for lt in range(LT):
                        qr = dwork.tile([P, C], F32, name=f"qr{lt}", tag="qr",
                                        bufs=8)
                        nc.sync.dma_start(qr, q_in[lt * P:(lt + 1) * P, :])
                        qrs.append(qr)
                    for lt in range(LT):
                        yp = psum_y.tile([P, C], F32, tag="yp")
                        ysb = dwork.tile([P, C], F32, tag="ysb")
                        st = dsmall.tile([P, 2, 6], F32, tag="st")
                        for ch in range(NH):
                            for dt in range(DT):
                                nc.tensor.matmul(
                                    yp[:, ch * 512:(ch + 1) * 512],
                                    oT[:, dt, lt * P:(lt + 1) * P],
                                    WoT[:, ch * 4:(ch + 1) * 4, dt, :],
                                    start=(dt == 0), stop=False)
                            nc.tensor.matmul(
                                yp[:, ch * 512:(ch + 1) * 512],
                                ones_row[0:1, :],
                                bo_bf[0:1, ch * 512:(ch + 1) * 512],
                                start=False, stop=True)
                            # evict + stats per 512-half while the other half runs
                            nc.vector.tensor_add(ysb[:, ch * 512:(ch + 1) * 512],
                                                 yp[:, ch * 512:(ch + 1) * 512],
                                                 qrs[lt][:, ch * 512:(ch + 1) * 512])
                            nc.vector.bn_stats(st[:, ch, :],
                                               ysb[:, ch * 512:(ch + 1) * 512])
                        mv = dsmall.tile([P, 2], F32, tag="mv")
                        nc.vector.bn_aggr(mv, st)
                        rstd = dsmall.tile([P, 1], F32, tag="rstd")
                        nc.scalar.activation(rstd, mv[:, 1:2], Sqrt,
                                             bias=eps_sb[:, 0:1])
                        nc.vector.reciprocal(rstd, rstd)
                        nmr = dsmall.tile([P, 1], F32, tag="nmr")
                        nc.vector.tensor_mul(nmr, mv[:, 0:1], rstd)
                        nc.vector.tensor_scalar_mul(nmr, nmr, -1.0)
                        yn = dwork.tile([P, C], F32, tag="yn")
                        nc.scalar.activation(yn, ysb, Identity, bias=nmr[:, 0:1],
                                             scale=rstd[:, 0:1])
                        if apply_gb:
                            nc.vector.tensor_mul(yn, yn, gamma_b)
                            nc.gpsimd.tensor_add(yn, yn, beta_b)
                        nc.sync.dma_start(y_out[lt * P:(lt + 1) * P, :], yn)

    nc.compile()
    return nc


def _get_nc(apply_gb):
    key = ("nc", apply_gb)
    if key not in _CACHE:
        _CACHE[key] = build(apply_gb)
    return _CACHE[key]


def kernel(**inputs) -> np.ndarray:
    global LAST_RESULT
    gamma = np.asarray(inputs["gamma"], dtype=np.float32)
    beta = np.asarray(inputs["beta"], dtype=np.float32)
    apply_gb = not (np.all(gamma == 1.0) and np.all(beta == 0.0))
    nc = _get_nc(apply_gb)
    q = np.ascontiguousarray(np.asarray(inputs["q"], dtype=np.float32))
    k = np.ascontiguousarray(np.asarray(inputs["k"], dtype=np.float32))
    v = np.ascontiguousarray(np.asarray(inputs["v"], dtype=np.float32))
    mask = np.ascontiguousarray(np.asarray(inputs["key_padding_mask"]).astype(np.uint8))
    shared = {
        name: np.ascontiguousarray(np.asarray(inputs[name], dtype=np.float32))
        for name in ("Wq", "bq", "Wk", "bk", "Wv", "bv", "Wo", "bo", "gamma", "beta")
    }
    in_maps = []
    for b in range(B):
        m = {"q": q[b], "k": k[b], "v": v[b], "key_padding_mask": mask[b]}
        m.update(shared)
        in_maps.append(m)
    LAST_RESULT = run_bass_kernel_spmd(nc, in_maps, core_ids=list(range(B)), trace=TRACE)
    return np.stack([r["y"] for r in LAST_RESULT.results], axis=0)


# revision 3
# speedup vs baseline: 1.3180x; 1.3180x over previous
"""Trainium2 Bass kernel for nn_CrossAttention (B=8, L=1024, QD=1024, KVD=768, H=16).

Sharding: data-parallel over batch across the 8 NeuronCores (1 batch row each).
Per-core pipeline (all bf16 matmuls, fp32 accumulation / residual / layernorm):
  A) prologue with NO DRAM bounce and NO DMA transposes: fp32->bf16 DMA-cast
     HBM->SBUF on SWDGE into natural row-tile chunks, then PE transposes
     (~56ns per 128x128 block in a stream) into bf16 PSUM banks, evicted per
     row-tile on DVE into lt-major transposed layouts [P, rowtile, ct, 128].
     Cuts HBM traffic ~46MB -> ~32MB/core and starts B1 at ~20us not ~85us.
  B) projections: qhT/khT (transposed world, per-partition bias via
     tensor_scalar), vh natural with bias added on DVE during psum eviction
     (bv broadcast tile). B1 is lh-outer so it can start on half of qT.
  C) attention per head pair: scoresT = khT.T @ qhT, exp with mask+scale folded
     into the ACT pass, attnV with [ones|vh] stationary giving psum rows 0:64 =
     replicated denominator and rows 64:128 = o; fast approx reciprocal +
     multiply on DVE. attnV lags one pair behind scores+exp. Wo's PE
     transposes share the b3 PSUM slot mid-attention.
  D) out-projection from oT stationary + rank-1 bias, fp32 residual + layernorm
     with per-512-half eviction/bn_stats to shorten the tail.
"""

import numpy as np

import concourse.bass as bass
import concourse.mybir as mybir
import concourse.tile as tile
from concourse import bacc
from concourse.bass_utils import run_bass_kernel_spmd
from concourse.masks import make_identity

F32 = mybir.dt.float32
BF16 = mybir.dt.bfloat16
U8 = mybir.dt.uint8

B = 8
L = 1024
C = 1024      # QD
KV = 768      # KVD
H = 16
DH = 64
P = 128
LT = L // P          # 8 l-tiles
CT = C // P          # 8 contraction tiles (model dim)
KT = KV // P         # 6 contraction tiles (kv dim)
DT = C // P          # 8 d-tiles
NH = C // 512        # 2 free-dim halves (N=512 per PSUM bank)
SCALE = DH ** -0.5
EPS = 1e-5
MASK_NEG = -30000.0

Exp = mybir.ActivationFunctionType.Exp
Sqrt = mybir.ActivationFunctionType.Sqrt
Identity = mybir.ActivationFunctionType.Identity
MULT = mybir.AluOpType.mult
ADD = mybir.AluOpType.add

TRACE = False
LAST_RESULT = None
_CACHE = {}


def _bcast_ap(handle, parts):
    apx = handle[:]
    return bass.AP(tensor=apx.tensor, offset=apx.offset,
                   ap=[[0, parts]] + [list(x) for x in apx.ap])


def _rowtiles(hnd, r0, nt, cols):
    # DRAM AP [p, j, c] = hnd[r0 + j*P + p, c] for j in [0, nt)
    return hnd[r0:r0 + nt * P, :].rearrange("(j p) c -> p j c", p=P)


def build(apply_gb=False):
    nc = bacc.Bacc("TRN2", target_bir_lowering=False)

    q_in = nc.dram_tensor("q", [L, C], F32, kind="ExternalInput")
    k_in = nc.dram_tensor("k", [L, KV], F32, kind="ExternalInput")
    v_in = nc.dram_tensor("v", [L, KV], F32, kind="ExternalInput")
    m_in = nc.dram_tensor("key_padding_mask", [L], U8, kind="ExternalInput")
    wq_in = nc.dram_tensor("Wq", [C, C], F32, kind="ExternalInput")
    bq_in = nc.dram_tensor("bq", [C], F32, kind="ExternalInput")
    wk_in = nc.dram_tensor("Wk", [C, KV], F32, kind="ExternalInput")
    bk_in = nc.dram_tensor("bk", [C], F32, kind="ExternalInput")
    wv_in = nc.dram_tensor("Wv", [C, KV], F32, kind="ExternalInput")
    bv_in = nc.dram_tensor("bv", [C], F32, kind="ExternalInput")
    wo_in = nc.dram_tensor("Wo", [C, C], F32, kind="ExternalInput")
    bo_in = nc.dram_tensor("bo", [C], F32, kind="ExternalInput")
    gamma_in = nc.dram_tensor("gamma", [C], F32, kind="ExternalInput")
    beta_in = nc.dram_tensor("beta", [C], F32, kind="ExternalInput")
    y_out = nc.dram_tensor("y", [L, C], F32, kind="ExternalOutput")

    with tile.TileContext(nc) as tc:
        with (
            tc.tile_pool(name="persist", bufs=1) as persist,
            tc.tile_pool(name="cst", bufs=1) as cst,
            tc.tile_pool(name="wostg", bufs=1) as wostg,
            tc.tile_pool(name="poolV", bufs=1) as poolV,
        ):
            # persistent projection outputs
            qhT = persist.tile([P, DT, L], BF16)          # d on partitions
            khT = persist.tile([P, DT, L], BF16)
            vh_aug = persist.tile([P, LT, H * P], BF16)   # per m-tile: 16x[64 ones | 64 vh]
            WoT = persist.tile([P, DT, CT, P], BF16)      # lt-major transposed Wo

            vT = poolV.tile([P, LT, KT, P], BF16)
            WvT = poolV.tile([P, DT, KT, P], BF16)
            wo_nat = [wostg.tile([P, 4, C], BF16, name=f"st_wo{ch}")
                      for ch in range(2)]

            def transp_rowtile(psum_pool, dstT, nat_ap, ctn, tag, ident):
                # nat_ap: [128, ctn*128] natural row-tile; writes dstT (same
                # jt slot) [P, ctn, P] via ctn PE transposes + one DVE evict.
                ps = psum_pool.tile([P, ctn, P], BF16, tag=tag)
                for ct in range(ctn):
                    nc.tensor.transpose(ps[:, ct, :],
                                        nat_ap[:, ct * P:(ct + 1) * P], ident)
                return ps

            with tc.tile_pool(name="poolK", bufs=1) as poolK:
                kT = poolK.tile([P, LT, KT, P], BF16)
                WkT = poolK.tile([P, DT, KT, P], BF16)

                with (
                    tc.tile_pool(name="stg", bufs=3) as stg,
                    tc.tile_pool(name="poolQ", bufs=1) as poolQ,
                    tc.tile_pool(name="psum_b", bufs=2, space="PSUM") as psum_b,
                    tc.tile_pool(name="pst", bufs=2, space="PSUM") as pst,
                ):
                    qT = poolQ.tile([P, LT, CT, P], BF16)
                    WqT = poolQ.tile([P, DT, CT, P], BF16)

                    # tiny consts first (they gate B evictions / first exps)
                    bq_sb = cst.tile([P, DT], F32)
                    nc.gpsimd.dma_start(bq_sb, bq_in[:].rearrange("(t p) -> p t", p=P))
                    bk_sb = cst.tile([P, DT], F32)
                    nc.gpsimd.dma_start(bk_sb, bk_in[:].rearrange("(t p) -> p t", p=P))
                    mask_u8 = cst.tile([P, LT], U8)
                    nc.gpsimd.dma_start(mask_u8, m_in[:].rearrange("(t p) -> p t", p=P))
                    mask_bias = cst.tile([P, LT], F32)
                    nc.vector.tensor_copy(mask_bias, mask_u8)
                    nc.vector.tensor_scalar(mask_bias, mask_bias, -MASK_NEG, MASK_NEG,
                                            MULT, ADD)
                    ident = cst.tile([P, P], BF16)
                    make_identity(nc, ident)
                    ones_row = cst.tile([1, P], BF16)
                    nc.vector.memset(ones_row, 1.0)
                    eps_sb = cst.tile([P, 1], F32)
                    nc.vector.memset(eps_sb, EPS)
                    bvb = cst.tile([P, C], F32)
                    nc.gpsimd.dma_start(bvb, _bcast_ap(bv_in, P))
                    bo_bf = cst.tile([1, C], BF16)
                    nc.gpsimd.dma_start(bo_bf, bo_in[:].rearrange("(a c) -> a c", a=1))
                    if apply_gb:
                        gamma_b = cst.tile([P, C], F32)
                        nc.gpsimd.dma_start(gamma_b, _bcast_ap(gamma_in, P))
                        beta_b = cst.tile([P, C], F32)
                        nc.gpsimd.dma_start(beta_b, _bcast_ap(beta_in, P))
                    else:
                        gamma_b = beta_b = None

                    # ---- loads: DMA-cast fp32->bf16 on SWDGE (gpsimd) in
                    # 4-rowtile chunks, natural layout.
                    nats = {}

                    def load_mat(nm, hnd, rows, cols, pool=stg):
                        tiles = []
                        for ch in range(rows // P // 4):
                            st = pool.tile([P, 4, cols], BF16,
                                           name=f"st_{nm}{ch}", tag=f"stg")
                            nc.gpsimd.dma_start(st, _rowtiles(hnd, ch * 4 * P, 4, cols))
                            tiles.append(st)
                        nats[nm] = tiles

                    load_mat("wq", wq_in, C, C)
                    load_mat("q", q_in, L, C)
                    load_mat("wk", wk_in, C, KV)
                    load_mat("k", k_in, L, KV)
                    load_mat("wv", wv_in, C, KV)
                    load_mat("v", v_in, L, KV)
                    for ch in range(2):
                        nc.gpsimd.dma_start(wo_nat[ch],
                                            _rowtiles(wo_in, ch * 4 * P, 4, C))

                    def transp_mat(nm, dstT, ctn, jts):
                        for jt in jts:
                            ps = transp_rowtile(pst, dstT,
                                                nats[nm][jt // 4][:, jt % 4, :],
                                                ctn, "pst", ident)
                            nc.vector.tensor_copy(dstT[:, jt, :, :], ps)

                    # ---- PE stream: transposes interleaved with B1/B2
                    transp_mat("wq", WqT, CT, range(8))
                    transp_mat("q", qT, CT, range(4))

                    def b_proj(dst, wT, xT, ctn, bias, lh, psum_pool):
                        for dt in range(DT):
                            ps = psum_pool.tile([P, 512], F32, tag="ps")
                            for ct in range(ctn):
                                nc.tensor.matmul(ps, wT[:, dt, ct, :],
                                                 xT[:, lh * 4:(lh + 1) * 4, ct, :],
                                                 start=(ct == 0), stop=(ct == ctn - 1))
                            nc.vector.tensor_scalar_add(
                                dst[:, dt, lh * 512:(lh + 1) * 512], ps,
                                bias[:, dt:dt + 1])

                    b_proj(qhT, WqT, qT, CT, bq_sb, 0, psum_b)   # B1 lh=0
                    transp_mat("q", qT, CT, range(4, 8))
                    b_proj(qhT, WqT, qT, CT, bq_sb, 1, psum_b)   # B1 lh=1
                    transp_mat("wk", WkT, KT, range(8))
                    transp_mat("k", kT, KT, range(4))
                    b_proj(khT, WkT, kT, KT, bk_sb, 0, psum_b)   # B2 lh=0
                    transp_mat("k", kT, KT, range(4, 8))
                    b_proj(khT, WkT, kT, KT, bk_sb, 1, psum_b)   # B2 lh=1
                    transp_mat("wv", WvT, KT, range(8))
                    transp_mat("v", vT, KT, range(8))

            with tc.tile_pool(name="late", bufs=1) as late:
                oT = late.tile([P, DT, L], BF16)

                # ---------------- attention, with B3 (vh projection)
                # interleaved into the first two pair slots
                with (
                    tc.tile_pool(name="ptp", bufs=26) as ptp,
                    tc.tile_pool(name="recp", bufs=4) as recp,
                    tc.tile_pool(name="psum_sc", bufs=2, space="PSUM") as psum_sc,
                    tc.tile_pool(name="psum_av", bufs=3, space="PSUM") as psum_av,
                    tc.tile_pool(name="psum_b3", bufs=1, space="PSUM") as psum_b3,
                ):
                    pts = {}

                    def scores_exp(pair):
                        for mt in range(LT):
                            sc = []
                            for hh in range(2):
                                s = psum_sc.tile([P, L], F32,
                                                 name=f"sc{pair}_{mt}_{hh}", tag="sc")
                                sc.append(s)
                                p0 = hh * DH
                                for lh in range(NH):
                                    nc.tensor.matmul(
                                        s[:, lh * 512:(lh + 1) * 512],
                                        khT[p0:p0 + DH, pair, mt * P:(mt + 1) * P],
                                        qhT[p0:p0 + DH, pair, lh * 512:(lh + 1) * 512],
                                        start=True, stop=True)
                            for hh in range(2):
                                pt = ptp.tile([P, L], BF16,
                                              name=f"pt{pair}_{mt}_{hh}", tag="pt")
                                pts[(pair, mt, hh)] = pt
                                nc.scalar.activation(pt, sc[hh], Exp,
                                                     bias=mask_bias[:, mt:mt + 1],
                                                     scale=SCALE)

                    def b3_chunk(mts):
                        for mt in mts:
                            for dh2 in range(NH):
                                ps = psum_b3.tile([P, 512], F32, tag="ps3")
                                for ct in range(KT):
                                    nc.tensor.matmul(
                                        ps, vT[:, mt, ct, :],
                                        WvT[:, dh2 * 4:(dh2 + 1) * 4, ct, :],
                                        start=(ct == 0), stop=(ct == KT - 1))
                                dst = vh_aug[:, mt, :].rearrange(
                                    "p (h x) -> p h x", x=P)
                                dst = dst[:, dh2 * 8:(dh2 + 1) * 8, DH:P]
                                bvs = bvb[:, dh2 * 512:(dh2 + 1) * 512].rearrange(
                                    "p (h d) -> p h d", d=DH)
                                nc.vector.tensor_add(
                                    dst, ps[:].rearrange("p (h d) -> p h d", d=DH),
                                    bvs)

                    def attnv(pair):
                        for hh in range(2):
                            h = 2 * pair + hh
                            avs = [psum_av.tile([P, 512], F32,
                                                name=f"av{pair}_{hh}_{lh}",
                                                tag="av")
                                   for lh in range(NH)]
                            # mt-outer: both l-halves reuse each vh stationary load
                            for mt in range(LT):
                                for lh in range(NH):
                                    nc.tensor.matmul(
                                        avs[lh],
                                        vh_aug[:, mt, h * P:(h + 1) * P],
                                        pts[(pair, mt, hh)][:, lh * 512:(lh + 1) * 512],
                                        start=(mt == 0), stop=(mt == LT - 1))
                            for lh in range(NH):
                                av = avs[lh]
                                rec = recp.tile([P, 512], F32,
                                                name=f"rec{pair}_{hh}_{lh}",
                                                tag="rec")
                                nc.vector.reciprocal_approx_fast(rec[0:DH, :],
                                                                 av[0:DH, :])
                                nc.vector.tensor_mul(
                                    oT[hh * DH:(hh + 1) * DH, pair,
                                       lh * 512:(lh + 1) * 512],
                                    av[DH:P, :], rec[0:DH, :])
                            for mt in range(LT):
                                del pts[(pair, mt, hh)]

                    nc.vector.memset(vh_aug[:], 1.0)
                    scores_exp(0)
                    b3_chunk(range(0, 4))
                    scores_exp(1)
                    b3_chunk(range(4, 8))
                    attnv(0)
                    scores_exp(2)
                    # Wo PE transposes mid-attention, sharing the b3 psum slot
                    for jt in range(8):
                        ps = psum_b3.tile([P, CT, P], BF16, tag="ps3")
                        for ct in range(CT):
                            nc.tensor.transpose(
                                ps[:, ct, :],
                                wo_nat[jt // 4][:, jt % 4, ct * P:(ct + 1) * P],
                                ident)
                        nc.vector.tensor_copy(WoT[:, jt, :, :], ps)
                    attnv(1)
                    for pair in range(3, H // 2):
                        scores_exp(pair)
                        attnv(pair - 1)
                    attnv(H // 2 - 1)

                # ---------------- out-projection + residual + layernorm
                with (
                    tc.tile_pool(name="dwork", bufs=3) as dwork,
                    tc.tile_pool(name="dsmall", bufs=8) as dsmall,
                    tc.tile_pool(name="psum_y", bufs=3, space="PSUM") as psum_y,
                ):
                    qrs = []
                    for lt in range(LT):
                        qr = dwork.tile([P, C], F32, name=f"qr{lt}", tag="qr",
                                        bufs=8)
                        nc.sync.dma_start(qr, q_in[lt * P:(lt + 1) * P, :])
                        qrs.append(qr)
                    for lt in range(LT):
                        yp = psum_y.tile([P, C], F32, tag="yp")
                        ysb = dwork.tile([P, C], F32, tag="ysb")
                        st = dsmall.tile([P, 2, 6], F32, tag="st")
                        for ch in range(NH):
                            for dt in range(DT):
                                nc.tensor.matmul(
                                    yp[:, ch * 512:(ch + 1) * 512],
                                    oT[:, dt, lt * P:(lt + 1) * P],
                                    WoT[:, ch * 4:(ch + 1) * 4, dt, :],
                                    start=(dt == 0), stop=False)
                            nc.tensor.matmul(
                                yp[:, ch * 512:(ch + 1) * 512],
                                ones_row[0:1, :],
                                bo_bf[0:1, ch * 512:(ch + 1) * 512],
                                start=False, stop=True)
                            # evict + stats per 512-half while the other half runs
                            nc.vector.tensor_add(ysb[:, ch * 512:(ch + 1) * 512],
                                                 yp[:, ch * 512:(ch + 1) * 512],
                                                 qrs[lt][:, ch * 512:(ch + 1) * 512])
                            nc.vector.bn_stats(st[:, ch, :],
                                               ysb[:, ch * 512:(ch + 1) * 512])
                        mv = dsmall.tile([P, 2], F32, tag="mv")
                        nc.vector.bn_aggr(mv, st)
                        rstd = dsmall.tile([P, 1], F32, tag="rstd")
                        nc.scalar.activation(rstd, mv[:, 1:2], Sqrt,
                                             bias=eps_sb[:, 0:1])
                        nc.vector.reciprocal(rstd, rstd)
                        nmr = dsmall.tile([P, 1], F32, tag="nmr")
                        nc.vector.tensor_mul(nmr, mv[:, 0:1], rstd)
                        nc.vector.tensor_scalar_mul(nmr, nmr, -1.0)
                        yn = dwork.tile([P, C], F32, tag="yn")
                        nc.scalar.activation(yn, ysb, Identity, bias=nmr[:, 0:1],
                                             scale=rstd[:, 0:1])
                        if apply_gb:
                            nc.vector.tensor_mul(yn, yn, gamma_b)
                            nc.gpsimd.tensor_add(yn, yn, beta_b)
                        nc.sync.dma_start(y_out[lt * P:(lt + 1) * P, :], yn)

    nc.compile()
    return nc


def _get_nc(apply_gb):
    key = ("nc", apply_gb)
    if key not in _CACHE:
        _CACHE[key] = build(apply_gb)
    return _CACHE[key]


def kernel(**inputs) -> np.ndarray:
    global LAST_RESULT
    gamma = np.asarray(inputs["gamma"], dtype=np.float32)
    beta = np.asarray(inputs["beta"], dtype=np.float32)
    apply_gb = not (np.all(gamma == 1.0) and np.all(beta == 0.0))
    nc = _get_nc(apply_gb)
    q = np.ascontiguousarray(np.asarray(inputs["q"], dtype=np.float32))
    k = np.ascontiguousarray(np.asarray(inputs["k"], dtype=np.float32))
    v = np.ascontiguousarray(np.asarray(inputs["v"], dtype=np.float32))
    mask = np.ascontiguousarray(np.asarray(inputs["key_padding_mask"]).astype(np.uint8))
    shared = {
        name: np.ascontiguousarray(np.asarray(inputs[name], dtype=np.float32))
        for name in ("Wq", "bq", "Wk", "bk", "Wv", "bv", "Wo", "bo", "gamma", "beta")
    }
    in_maps = []
    for b in range(B):
        m = {"q": q[b], "k": k[b], "v": v[b], "key_padding_mask": mask[b]}
        m.update(shared)
        in_maps.append(m)
    LAST_RESULT = run_bass_kernel_spmd(nc, in_maps, core_ids=list(range(B)), trace=TRACE)
    return np.stack([r["y"] for r in LAST_RESULT.results], axis=0)
